# revision 7
# baseline (speedup 1.0000x reference)
"""Trainium2 Bass kernel for the BenesBlock problem.

Row-sharded across 8 NeuronCores: each core owns L/(2*8) row-pairs per switch
stage.  Per stage: local GEMM1 -> tiny stats AllReduce (layernorm over axis 0
needs global per-column mean/var) -> leaky-relu -> local GEMM2 -> residual ->
AllGather of each core's output shard.  The Benes bit-rotation shuffles are
folded into per-core gather DMAs with partition-id-dependent offsets.
"""

import sys

sys.path.insert(0, "/opt/trn_rl_repo")

import numpy as np

import concourse.bass as bass
import concourse.bacc as bacc
import concourse.mybir as mybir
import concourse.tile as tile
from concourse.bass_interp import get_hw_module
from concourse.bass_utils import run_bass_kernel_spmd

F32 = mybir.dt.float32
BF16 = mybir.dt.bfloat16
NP_BF16 = mybir.dt.np(BF16)
ALU = mybir.AluOpType
ACTF = mybir.ActivationFunctionType

C = 8  # cores

RESIDUAL_WEIGHT = 0.9
CANDIDATE_WEIGHT = float(np.sqrt(1.0 - RESIDUAL_WEIGHT**2) * 0.25)
EPS = 1e-6


def build_program(L, NU, nf, nr):
    """Build the SPMD Bass program. Returns the bacc module (compiled)."""
    R = L // (2 * C)  # local pairs per core (free dim of all tiles)
    DIN = 2 * NU
    DHID = 4 * NU
    KT1 = DIN // 128  # v feature tiles == GEMM1 k-tiles == GEMM2 m-tiles
    MT1 = DHID // 128  # hidden tiles == GEMM1 m-tiles == GEMM2 k-tiles
    KTH = KT1 // 2  # tiles per NU half
    RH = R // 2
    INV_N = 1.0 / (L // 2)
    nstages = nf + nr + 1

    nc = bacc.Bacc(
        "TRN2",
        target_bir_lowering=False,
        debug=False,
        enable_asserts=False,
        num_devices=C,
    )

    # ---- kernel I/O ----
    v0 = nc.dram_tensor("v0", [DIN, R], F32, kind="ExternalInput")
    wts = {}
    for tag in ("f", "r", "m"):
        wts[tag] = dict(
            w1=nc.dram_tensor(f"w1{tag}", [DIN, DHID], BF16, kind="ExternalInput"),
            w2=nc.dram_tensor(f"w2{tag}", [DHID, DIN], BF16, kind="ExternalInput"),
            srs=nc.dram_tensor(f"srs{tag}", [128, KT1], F32, kind="ExternalInput"),
            cb2=nc.dram_tensor(f"cb2{tag}", [128, KT1], F32, kind="ExternalInput"),
        )
    zout = nc.dram_tensor("zout", [DIN, R], F32, kind="ExternalOutput")

    rg = [list(range(C))]

    with tile.TileContext(nc, trace_sim=False) as tc:
        with (
            tc.tile_pool(name="res", bufs=1) as res,       # resident: weights, srs/cb2
            tc.tile_pool(name="sta", bufs=1) as stap,      # v staging (f32)
            tc.tile_pool(name="vb", bufs=1) as vbp,        # v bf16
            tc.tile_pool(name="hb", bufs=1) as hbp,        # h bf16
            tc.tile_pool(name="gp", bufs=1) as gpool,      # g bf16
            tc.tile_pool(name="sq", bufs=2) as sqp,        # ttr dump
            tc.tile_pool(name="zp", bufs=3) as zp,         # z out tiles
            tc.tile_pool(name="st", bufs=2) as stp,        # small stats tiles
            tc.tile_pool(name="hps", bufs=3, space="PSUM") as hps,
            tc.tile_pool(name="cps", bufs=3, space="PSUM") as cps,
            tc.tile_pool(name="dram", bufs=1, space="DRAM") as dram,
        ):
            pid = nc.sync.partition_id()

            # ---- internal DRAM ----
            Zbuf = dram.tile([DIN, R], F32, tag="Zbuf", name="Zbuf")
            Gbuf = [
                dram.tile([C * DIN, R], F32, tag=f"Gbuf{i}", name=f"Gbuf{i}",
                          addr_space="Shared")
                for i in range(nstages - 1)
            ]
            statin = dram.tile([DHID, 2], F32, tag="statin", name="statin")
            statout = [
                dram.tile([DHID, 2], F32, tag=f"statout{i}", name=f"statout{i}",
                          addr_space="Shared")
                for i in range(nstages)
            ]

            # ---- resident weights: set A holds f (later m), set B holds r ----
            def load_wset(w1_tiles, w2_tiles, src):
                for k in range(KT1):
                    nc.sync.dma_start(
                        out=w1_tiles[k][:], in_=src["w1"][128 * k : 128 * (k + 1), :]
                    )
                for k in range(MT1):
                    nc.sync.dma_start(
                        out=w2_tiles[k][:], in_=src["w2"][128 * k : 128 * (k + 1), :]
                    )

            w1A = [res.tile([128, DHID], BF16, tag=f"w1A{k}", name=f"w1A{k}") for k in range(KT1)]
            w2A = [res.tile([128, DIN], BF16, tag=f"w2A{k}", name=f"w2A{k}") for k in range(MT1)]
            w1B = [res.tile([128, DHID], BF16, tag=f"w1B{k}", name=f"w1B{k}") for k in range(KT1)]
            w2B = [res.tile([128, DIN], BF16, tag=f"w2B{k}", name=f"w2B{k}") for k in range(MT1)]
            load_wset(w1A, w2A, wts["f"])
            load_wset(w1B, w2B, wts["r"])
            sc = {}
            for tag in ("f", "r", "m"):
                sc[tag] = dict(
                    srs=res.tile([128, KT1], F32, tag=f"srs{tag}", name=f"srs{tag}_sb"),
                    cb2=res.tile([128, KT1], F32, tag=f"cb2{tag}", name=f"cb2{tag}_sb"),
                )
                nc.sync.dma_start(out=sc[tag]["srs"][:], in_=wts[tag]["srs"][:, :])
                nc.sync.dma_start(out=sc[tag]["cb2"][:], in_=wts[tag]["cb2"][:, :])

            def stage(s):
                """One switch stage."""
                if s < nf:
                    w1, w2, scs = w1A, w2A, sc["f"]
                elif s < nf + nr:
                    w1, w2, scs = w1B, w2B, sc["r"]
                else:
                    w1, w2, scs = w1A, w2A, sc["m"]

                gmode = "in" if s == 0 else ("fwd" if s <= nf else "rev")
                Gin = Gbuf[s - 1] if s > 0 else None
                Gout = Gbuf[s] if s < nstages - 1 else None
                phi_tau = nf <= s < nf + nr  # write G in tau (shuffle-blocked) order
                last = s == nstages - 1

                # ---- gather v (f32) into staging tiles ----
                sta = [stap.tile([128, R], F32, tag=f"sta{t}", name=f"sta{t}_{s}") for t in range(KT1)]
                interleaved = gmode == "fwd"
                for t in range(KT1):
                    tt = t % KTH
                    bot = t >= KTH
                    if gmode == "in":
                        nc.sync.dma_start(
                            out=sta[t][:], in_=v0[128 * t : 128 * (t + 1), :]
                        )
                    elif gmode == "fwd":
                        # sta col (RH*s2 + mh) <- G[DIN*ch + NU*s2 + 128*tt + p, RH*e + mh]
                        for s2 in range(2):
                            rowbase = (
                                DIN * ((pid // 2) + (C // 2 if bot else 0))
                                + NU * s2
                                + 128 * tt
                            )
                            nc.sync.dma_start(
                                out=sta[t][:, RH * s2 : RH * (s2 + 1)],
                                in_=Gin[bass.ds(rowbase, 128), bass.ds(RH * (pid % 2), RH)],
                            )
                    else:
                        # direct linear: v[p, RH*H + mm] <-
                        #   G[DIN*(2*(pid%4) + H) + NU*(pid//4) + 128*tt + p, RH*beta + mm]
                        beta = 1 if bot else 0
                        for H in range(2):
                            rowbase = (
                                DIN * (2 * (pid % (C // 2)) + H)
                                + NU * (pid // (C // 2))
                                + 128 * tt
                            )
                            nc.sync.dma_start(
                                out=sta[t][:, RH * H : RH * (H + 1)],
                                in_=Gin[
                                    bass.ds(rowbase, 128),
                                    RH * beta : RH * (beta + 1),
                                ],
                            )

                # ---- cast to bf16 (undo column blocking for fwd) ----
                vb = [vbp.tile([128, R], BF16, tag=f"vb{t}", name=f"vb{t}_{s}") for t in range(KT1)]
                for t in range(KT1):
                    if interleaved:
                        dst = vb[t][:, :].rearrange("p (mh ml) -> p ml mh", ml=2)
                        nc.vector.tensor_copy(dst, sta[t][:, :])
                    else:
                        nc.vector.tensor_copy(vb[t][:, :], sta[t][:, :])

                # ---- GEMM1 + stats ----
                hb = [hbp.tile([128, R], BF16, tag=f"hb{m}", name=f"hb{m}_{s}") for m in range(MT1)]
                for m in range(MT1):
                    hp = hps.tile([128, R], F32, tag="hp", name=f"hp{m}_{s}")
                    for k in range(KT1):
                        nc.tensor.matmul(
                            hp[:],
                            w1[k][:, 128 * m : 128 * (m + 1)],
                            vb[k][:],
                            start=(k == 0),
                            stop=(k == KT1 - 1),
                        )
                    st = stp.tile([128, 2], F32, tag=f"st{m}", name=f"st{m}_{s}")
                    nc.scalar.activation(hb[m][:], hp[:], ACTF.Copy)
                    sq = sqp.tile([128, R], BF16, tag="sq", name=f"sq{m}_{s}")
                    nc.vector.reduce_sum(
                        st[:, 0:1], hb[m][:], axis=mybir.AxisListType.X
                    )
                    nc.vector.tensor_mul(sq[:], hb[m][:], hb[m][:])
                    nc.vector.reduce_sum(
                        st[:, 1:2], sq[:], axis=mybir.AxisListType.X
                    )
                    nc.sync.dma_start(
                        out=statin[128 * m : 128 * (m + 1), :], in_=st[:]
                    )

                # ---- stats AllReduce + norm params ----
                nc.gpsimd.collective_compute(
                    "AllReduce", ALU.add, replica_groups=rg,
                    ins=[statin.opt()], outs=[statout[s].opt()],
                )
                gstat = stp.tile([128, MT1, 2], F32, tag="gstat", name=f"gstat_{s}")
                nc.sync.dma_start(
                    out=gstat[:],
                    in_=statout[s][:, :].rearrange("(t p) s -> p t s", p=128),
                )
                mean = stp.tile([128, MT1], F32, tag="mean", name=f"mean_{s}")
                var = stp.tile([128, MT1], F32, tag="var", name=f"var_{s}")
                rstd = stp.tile([128, MT1], F32, tag="rstd", name=f"rstd_{s}")
                negmb = stp.tile([128, MT1], F32, tag="negmb", name=f"negmb_{s}")
                nc.vector.tensor_scalar_mul(mean[:], gstat[:, :, 0], INV_N)
                nc.vector.tensor_scalar_mul(var[:], gstat[:, :, 1], INV_N)
                # var = E[h^2] - mean^2 + EPS
                nc.vector.scalar_tensor_tensor(
                    out=rstd[:], in0=mean[:], scalar=-1.0, in1=mean[:],
                    op0=ALU.mult, op1=ALU.mult,
                )  # rstd <- -mean^2 (scratch)
                nc.vector.tensor_add(var[:], var[:], rstd[:])
                nc.vector.tensor_scalar_add(var[:], var[:], EPS)
                nc.vector.reciprocal(var[:], var[:])  # var <- 1/(var+eps)
                nc.scalar.activation(rstd[:], var[:], ACTF.Sqrt)
                nc.vector.scalar_tensor_tensor(
                    out=negmb[:], in0=mean[:], scalar=-1.0, in1=rstd[:],
                    op0=ALU.mult, op1=ALU.mult,
                )

                # ---- normalize + leaky relu:  g = max(u, 0.2*u),  u = h*rstd - mean*rstd
                g = [gpool.tile([128, R], BF16, tag=f"g{m}", name=f"g{m}_{s}") for m in range(MT1)]
                for m in range(MT1):
                    nc.scalar.activation(
                        g[m][:], hb[m][:], ACTF.Identity,
                        scale=rstd[:, m : m + 1], bias=negmb[:, m : m + 1],
                    )
                    nc.vector.scalar_tensor_tensor(
                        out=g[m][:], in0=g[m][:], scalar=0.2, in1=g[m][:],
                        op0=ALU.mult, op1=ALU.max,
                    )

                # ---- GEMM2 + residual + sink ----
                for mo in range(KT1):
                    cp = cps.tile([128, R], F32, tag="cp", name=f"cp{mo}_{s}")
                    for k in range(MT1):
                        nc.tensor.matmul(
                            cp[:],
                            w2[k][:, 128 * mo : 128 * (mo + 1)],
                            g[k][:],
                            start=(k == 0),
                            stop=(k == MT1 - 1),
                        )
                    z = zp.tile([128, R], F32, tag="z", name=f"z{mo}_{s}")
                    if interleaved:
                        v_ap = sta[mo][:, :].rearrange("p (ul uh) -> p uh ul", uh=RH)
                    else:
                        v_ap = sta[mo][:, :].rearrange("p (uh ul) -> p uh ul", ul=2)
                    cp_ap = cp[:, :].rearrange("p (uh ul) -> p uh ul", ul=2)
                    if phi_tau and not last:
                        z_ap = z[:, :].rearrange("p (ul uh) -> p uh ul", uh=RH)
                    else:
                        z_ap = z[:, :].rearrange("p (uh ul) -> p uh ul", ul=2)
                    nc.vector.scalar_tensor_tensor(
                        out=z_ap, in0=v_ap, scalar=scs["srs"][:, mo : mo + 1],
                        in1=cp_ap, op0=ALU.mult, op1=ALU.add,
                    )
                    nc.vector.tensor_scalar_add(
                        z[:], z[:], scs["cb2"][:, mo : mo + 1]
                    )
                    if last:
                        nc.sync.dma_start(
                            out=zout[128 * mo : 128 * (mo + 1), :], in_=z[:]
                        )
                    else:
                        nc.sync.dma_start(
                            out=Zbuf[128 * mo : 128 * (mo + 1), :], in_=z[:]
                        )

                if not last:
                    nc.gpsimd.collective_compute(
                        "AllGather", ALU.bypass, replica_groups=rg,
                        ins=[Zbuf.opt()], outs=[Gout.opt()],
                    )

            for s in range(nstages):
                stage(s)
                if s == nf:
                    # refill set A with the mid-stage weights (overlaps r-epoch)
                    load_wset(w1A, w2A, wts["m"])

    nc.compile()
    nc.m = get_hw_module(nc.m)
    return nc


def host_inputs(inputs, L, NU, nf, nr):
    """Build the 8 per-core in_maps from the full problem inputs."""
    R = L // (2 * C)
    DIN = 2 * NU
    KT1 = DIN // 128

    x = np.asarray(inputs["x"], np.float32)
    shared = {}
    for tag in ("f", "r", "m"):
        w1 = np.asarray(inputs[f"w1_{tag}"], np.float32)
        w2 = np.asarray(inputs[f"w2_{tag}"], np.float32)
        rs = np.asarray(inputs[f"rs_{tag}"], np.float32)
        b2 = np.asarray(inputs[f"b2_{tag}"], np.float32)
        srs = 1.0 / (1.0 + np.exp(-rs))  # sigmoid
        srs2 = np.concatenate([srs, srs]).astype(np.float32)  # [DIN]
        cb2 = (CANDIDATE_WEIGHT * b2).astype(np.float32)  # [DIN]
        shared[f"w1{tag}"] = w1.astype(NP_BF16)
        shared[f"w2{tag}"] = (CANDIDATE_WEIGHT * w2).astype(NP_BF16)
        shared[f"srs{tag}"] = np.ascontiguousarray(srs2.reshape(KT1, 128).T)
        shared[f"cb2{tag}"] = np.ascontiguousarray(cb2.reshape(KT1, 128).T)

    in_maps = []
    for c in range(C):
        xc = x[2 * R * c : 2 * R * (c + 1)]  # [2R, NU]
        v0 = np.ascontiguousarray(
            xc.reshape(R, 2, NU).transpose(1, 2, 0).reshape(DIN, R)
        )
        in_maps.append({"v0": v0, **shared})
    return in_maps


def unshard(results, L, NU):
    R = L // (2 * C)
    y = np.zeros((L, NU), np.float32)
    for c in range(C):
        zc = results[c]["zout"]  # [DIN, R]
        blk = zc.reshape(2, NU, R).transpose(2, 0, 1).reshape(2 * R, NU)
        y[2 * R * c : 2 * R * (c + 1)] = blk
    return y


_PROG_CACHE = {}


def run(inputs, L=8192, NU=512, nf=12, nr=12, trace=False):
    key = (L, NU, nf, nr)
    if key not in _PROG_CACHE:
        _PROG_CACHE[key] = build_program(L, NU, nf, nr)
    nc = _PROG_CACHE[key]
    in_maps = host_inputs(inputs, L, NU, nf, nr)
    res = run_bass_kernel_spmd(nc, in_maps, list(range(C)), trace=trace)
    return unshard(res.results, L, NU), res


def kernel(**inputs) -> np.ndarray:
    out, _ = run(inputs, L=8192, NU=512, nf=12, nr=12)
    return out
